# revision 17
# baseline (speedup 1.0000x reference)
"""Trainium2 Bass kernel for nn_DGL_GCN (3-layer hetero GCN + MLP head).

Math (reference): 3x hetero layers
    h' = relu( sum_e segment_mean_e( h @ W_e.T + b_e ) )
then z = relu(fc1_w @ h3.flatten() + fc1_b); out = sigmoid(fc2_w @ z + fc2_b).

Key algebra: A_e @ (h @ W_e.T) == (A_e @ h) @ W_e.T, so each core
aggregates first (contraction over all 4096 src for its own 512 dst)
and transforms only its 512-dst aggregate -- no redundant per-core Wh.

A_e[src, dst] = cnt(src->dst)/max(deg(dst),1) is stored fp8 as
cnt * q_d with q_d = fp8(A_SCALE/deg_d) (exact for cnt in {1,2,4});
the residual per-(etype,dst) scale is applied at the PSUM drain.
A-multiply and fc1 run fp8 DoubleRow (2 fp8/cell, 2x rate).

Sharding: destination-node shards (512 dst/core, all 8 etypes on-core,
cross-etype sum in the transform PSUM). One AllGather of the fp8 h
shard per layer boundary. Etypes 5-7 stay SBUF-resident after layer 0
(refilled from HBM during the idle boundary window); their slot is
recycled as extra fc1 prefetch space at the tail. fc1 is
column-sharded over the flattened node*hidden dim; partial z
AllGathered, rank-summed on PE.
"""

import numpy as np
import ml_dtypes

N_OBJ = 4096
F_IN = 256
H = 256
C = 128
NE = 8
NCORES = 8
SHARD = N_OBJ // NCORES          # 512 dst nodes per core
NCH = 16                         # src chunks of 256 (DoubleRow K-tiles)
NCHP = 8                         # chunk pairs (A DMA batching)
R_RES = 3                        # etypes NE-R..NE-1 SBUF-resident
E_RES0 = NE - R_RES
FC1_NBLK = 32                    # fc1 blocks of 16 chunks (4096 flat k each)
FC1_BUFS = 8                     # fc1 prefetch ring depth (8 KB/partition)
N_SCAV_A = 2 * R_RES             # fc1 blocks parked in the dead ares slot
N_SCAV_G = 2                     # fc1 blocks parked in dead g slots

# chunk consumption order: evens first, odds second -- the odd half of
# the split AllGather lands while the even half is being consumed
ORDER = list(range(0, NCH, 2)) + list(range(1, NCH, 2))

BF16 = ml_dtypes.bfloat16
FP8 = ml_dtypes.float8_e4m3
H_SCALE = 16.0    # hidden state kept in S*h domain (relu commutes)
FC1_SCALE = 8192.0
A_SCALE = 4.0     # folded into the drain correction

_BASS_CACHE = {}

PASSES = ((0, 1, 2, 3), (4, 5, 6, 7))


def _split_drain_waits(nc, max_waits=1):
    # This walrus build accepts only one sync-wait command on an InstDrain;
    # Tile's tail drain waits on every active proc lane. Split into a chain
    # of single-wait drains.
    import copy
    import concourse.mybir as mybir

    for f in nc.m.functions:
        for bb in f.blocks:
            new_list = []
            for ins in bb.instructions:
                si = ins.sync_info
                if (
                    isinstance(ins, mybir.InstDrain)
                    and si is not None
                    and si.on_wait
                    and len(si.on_wait) > max_waits
                ):
                    waits = list(si.on_wait)
                    updates = list(si.on_update or [])
                    for i, w in enumerate(waits[:-1]):
                        d = copy.deepcopy(ins)
                        d.name = f"{ins.name}-sw{i}"
                        dsi = d.sync_info
                        dsi.on_wait = [w]
                        dsi.on_update = []
                        d.sync_info = dsi
                        new_list.append(d)
                        nc.inst_map[d.name] = d
                    si.on_wait = [waits[-1]]
                    si.on_update = updates
                    ins.sync_info = si
                new_list.append(ins)
            bb.instructions[:] = new_list


def _build_bass(has_bias=False):
    import concourse.bass as bass  # noqa: F401
    import concourse.tile as tile
    import concourse.mybir as mybir
    from concourse import bacc

    f32 = mybir.dt.float32
    bf16 = mybir.dt.bfloat16
    fp8 = mybir.dt.float8e4
    AF = mybir.ActivationFunctionType
    DR = mybir.MatmulPerfMode.DoubleRow
    ALU = mybir.AluOpType

    nc = bacc.Bacc(
        "TRN2", target_bir_lowering=False, debug=False, num_devices=NCORES
    )

    # ---- I/O (per-core values supplied via in_maps) ----
    G0 = nc.dram_tensor("g0", [128, NCH, 2, F_IN], fp8, kind="ExternalInput")
    ATP = nc.dram_tensor(
        "atp", [NE, NCHP, 128, 2, 2, SHARD], fp8, kind="ExternalInput"
    )
    CORR = nc.dram_tensor("corr", [128, NE, SHARD], bf16, kind="ExternalInput")
    WT = nc.dram_tensor("wt", [3, 128, NE * 2, H], bf16, kind="ExternalInput")
    FC1T = nc.dram_tensor(
        "fc1t", [FC1_NBLK, 128, NCH, 2, H], fp8, kind="ExternalInput"
    )
    FC1B = nc.dram_tensor("fc1b", [128, 2], f32, kind="ExternalInput")
    FC2T = nc.dram_tensor("fc2t", [128, 2 * C], bf16, kind="ExternalInput")
    FC2B = nc.dram_tensor("fc2b", [128, 1], f32, kind="ExternalInput")
    if has_bias:
        HBN = nc.dram_tensor("hbn", [128, 2, 4, H], bf16, kind="ExternalInput")
        HBT = nc.dram_tensor("hbt", [128, 2, SHARD], bf16, kind="ExternalInput")
    OUT = nc.dram_tensor("out", [C, 1], f32, kind="ExternalOutput")

    rg = [list(range(NCORES))]

    with tile.TileContext(nc) as tc:
        with (
            tc.tile_pool(name="wpool", bufs=1) as wpool,
            tc.tile_pool(name="gpool", bufs=2) as gpool,
            tc.tile_pool(name="arespool", bufs=1) as arespool,
            tc.tile_pool(name="atpool", bufs=8) as atpool,
            tc.tile_pool(name="aggpool", bufs=16) as aggpool,
            tc.tile_pool(name="hpool", bufs=2) as hpool,
            tc.tile_pool(name="fcpool", bufs=FC1_BUFS) as fcpool,
            tc.tile_pool(name="spool", bufs=2) as spool,
            tc.tile_pool(name="aggp", bufs=8, space="PSUM") as aggp,
            tc.tile_pool(name="dram", bufs=2, space="DRAM") as dram,
        ):
            # ---- initial g (feat, DoubleRow-interleaved natural layout) ----
            g = gpool.tile([128, NCH, 2, F_IN], fp8, tag="g", name="g_l0")
            nc.sync.dma_start(g[:], G0[:])

            ares = arespool.tile(
                [128, R_RES, NCHP, 2, 2, SHARD], fp8, tag="ares", name="ares"
            )

            corr_sb = wpool.tile([128, NE, SHARD], bf16)
            fc1b_sb = wpool.tile([128, 2], f32)
            fc2t_sb = wpool.tile([128, 2 * C], bf16)
            fc2b_sb = wpool.tile([128, 1], f32)
            ones8 = wpool.tile([NCORES, 1], f32)
            nc.gpsimd.memset(ones8[:], 1.0)
            wt_sb = [
                wpool.tile([128, NE * 2, H], bf16, tag=f"wt{l}", name=f"wt{l}")
                for l in range(3)
            ]
            if has_bias:
                hbn_sb = wpool.tile([128, 2, 4, H], bf16)
                hbt_sb = wpool.tile([128, 2, SHARD], bf16)

            # warmup collective, shape-identical to the layer AllGather: pays
            # the one-time ncfw/collective init hidden under layer-0 compute
            wuin = dram.tile([128, 4, F_IN], fp8, tag="agin", name="wuin")
            nc.gpsimd.dma_start(
                wuin[:], G0[:, 0:2, :, :].rearrange("p a i f -> p (a i) f")
            )
            wuout = dram.tile(
                [NCORES, 128, 4, F_IN], fp8, tag="agout", addr_space="Shared",
                name="wuout",
            )
            nc.gpsimd.collective_compute(
                "AllGather",
                ALU.bypass,
                replica_groups=rg,
                ins=[wuin.opt()],
                outs=[wuout.opt()],
            )
            # also warm the z-shaped gather (f32 [1, H]) -- a cold collective
            # of this shape cost ~15us on the critical tail
            wuzin = dram.tile([1, H], f32, tag="agzin", name="wuzin")
            nc.gpsimd.dma_start(
                wuzin[:].rearrange("a b -> (a b)"),
                FC1B[:].rearrange("p x -> (p x)"),
            )
            wuzout = dram.tile(
                [NCORES, 1, H], f32, tag="agzout", addr_space="Shared",
                name="wuzout",
            )
            nc.gpsimd.collective_compute(
                "AllGather",
                ALU.bypass,
                replica_groups=rg,
                ins=[wuzin.opt()],
                outs=[wuzout.opt()],
            )

            fc1_tiles = {}

            def issue_fc1(blk, ap=None):
                if ap is None:
                    ap = fcpool.tile(
                        [128, NCH, 2, H], fp8, tag="fc1", name=f"fc1_{blk}"
                    )
                nc.scalar.dma_start(ap[:], FC1T[blk])
                fc1_tiles[blk] = ap

            h3q = None
            for layer in range(3):
                aggT = {}
                for pi, pe in enumerate(PASSES):
                    # per-etype aggregate PSUMs for this pass
                    pg = {
                        (e, fh): aggp.tile(
                            [128, SHARD], f32, tag="agg",
                            name=f"pg_l{layer}_e{e}_f{fh}",
                        )
                        for e in pe
                        for fh in range(2)
                    }
                    for j in range(NCHP):
                        at_t = {}
                        for e in pe:
                            if layer > 0 and e >= E_RES0:
                                at_t[e] = ares[:, e - E_RES0, j]
                            else:
                                t = atpool.tile(
                                    [128, 2, 2, SHARD], fp8, tag="at",
                                    name=f"at_l{layer}_e{e}_c{j}",
                                )
                                nc.sync.dma_start(t[:], ATP[e, j])
                                at_t[e] = t
                        for sub in range(2):
                            ch = ORDER[2 * j + sub]
                            for fh in range(2):
                                lhsT = g[:, ch, :, fh * 128 : (fh + 1) * 128]
                                for e in pe:
                                    nc.tensor.matmul(
                                        pg[(e, fh)][:],
                                        lhsT=lhsT,
                                        rhs=at_t[e][:, sub],
                                        start=(j == 0 and sub == 0),
                                        stop=(j == NCHP - 1 and sub == 1),
                                        perf_mode=DR,
                                    )
                    # stagger the big resident loads behind layer-0 streaming
                    # (emitted BEFORE the drains/W-mults that consume them --
                    # Tile dependencies follow emission order)
                    if layer == 0 and pi == 0:
                        nc.sync.dma_start(corr_sb[:], CORR[:])
                        nc.sync.dma_start(wt_sb[0][:], WT[0])
                        if has_bias:
                            nc.sync.dma_start(hbn_sb[:], HBN[:])
                            nc.sync.dma_start(hbt_sb[:], HBT[:])
                    elif layer == 0 and pi == 1:
                        nc.sync.dma_start(wt_sb[1][:], WT[1])
                        nc.sync.dma_start(fc1b_sb[:], FC1B[:])
                        nc.sync.dma_start(fc2t_sb[:], FC2T[:])
                        nc.sync.dma_start(fc2b_sb[:], FC2B[:])
                    elif layer == 1 and pi == 0:
                        nc.sync.dma_start(wt_sb[2][:], WT[2])
                    # drain with per-(etype,dst) fp8-A correction
                    for e in pe:
                        for fh in range(2):
                            t = aggpool.tile(
                                [128, SHARD], bf16, tag="aggT",
                                name=f"aggT_l{layer}_e{e}_f{fh}",
                            )
                            nc.vector.scalar_tensor_tensor(
                                t[:], pg[(e, fh)][:], 1.0, corr_sb[:, e, :],
                                ALU.bypass, ALU.mult,
                            )
                            aggT[(e, fh)] = t

                if layer < 2:
                    # transform h_next[dst,:] = relu(sum_e agg_e @ W_e.T)
                    hsh = hpool.tile(
                        [128, 4, H], fp8, tag="hsh", name=f"hsh_l{layer}"
                    )
                    for dt in range(4):
                        ph = aggp.tile(
                            [128, H], f32, tag="agg",
                            name=f"ph_l{layer}_d{dt}",
                        )
                        for e in range(NE):
                            for fh in range(2):
                                nc.tensor.matmul(
                                    ph[:],
                                    lhsT=aggT[(e, fh)][
                                        :, dt * 128 : (dt + 1) * 128
                                    ],
                                    rhs=wt_sb[layer][:, e * 2 + fh, :],
                                    start=(e == 0 and fh == 0),
                                    stop=(e == NE - 1 and fh == 1),
                                )
                        if has_bias:
                            nc.vector.scalar_tensor_tensor(
                                ph[:], ph[:], 1.0, hbn_sb[:, layer, dt, :],
                                ALU.bypass, ALU.add,
                            )
                        nc.scalar.activation(hsh[:, dt, :], ph[:], AF.Relu)
                    agin = dram.tile(
                        [128, 4, H], fp8, tag="agin", name=f"agin_l{layer}"
                    )
                    nc.gpsimd.dma_start(agin[:], hsh[:])
                    agout = dram.tile(
                        [NCORES, 128, 4, H], fp8, tag="agout",
                        addr_space="Shared", name=f"agout_l{layer}",
                    )
                    nc.gpsimd.collective_compute(
                        "AllGather",
                        ALU.bypass,
                        replica_groups=rg,
                        ins=[agin.opt()],
                        outs=[agout.opt()],
                    )
                    g = gpool.tile(
                        [128, NCH, 2, F_IN], fp8, tag="g",
                        name=f"g_l{layer+1}",
                    )
                    for c in range(NCORES):
                        nc.sync.dma_start(
                            g[:, 2 * c : 2 * c + 2, :, :],
                            agout[c].rearrange("p (a i) f -> p a i f", a=2),
                        )
                    if layer == 0:
                        # refill the resident etypes from HBM on the scalar
                        # DGE (its queue is idle here) -- runs during the
                        # AllGather wait without blocking the sync queue
                        for e in range(E_RES0, NE):
                            for j in range(NCHP):
                                nc.scalar.dma_start(
                                    ares[:, e - E_RES0, j], ATP[e, j]
                                )
                else:
                    # final layer: produce h3.T (fc1 lhsT layout), fp8
                    # first: park extra fc1 blocks in the now-dead ares/g
                    # slots (their last readers were this layer's A-mults)
                    scav = arespool.tile(
                        [128, N_SCAV_A, NCH, 2, H], fp8, tag="ares",
                        name="fc1scav",
                    )
                    for i in range(N_SCAV_A):
                        issue_fc1(FC1_BUFS + i, ap=scav[:, i])
                    for i in range(N_SCAV_G):
                        gs = gpool.tile(
                            [128, NCH, 2, H], fp8, tag="g", name=f"fc1gs{i}"
                        )
                        issue_fc1(FC1_BUFS + N_SCAV_A + i, ap=gs)

                    h3q = hpool.tile([128, 2, SHARD], fp8, name="h3q")
                    for jh in range(2):
                        ph3 = aggp.tile(
                            [128, SHARD], f32, tag="agg", name=f"ph3_j{jh}"
                        )
                        for e in range(NE):
                            for fh in range(2):
                                nc.tensor.matmul(
                                    ph3[:],
                                    lhsT=wt_sb[2][
                                        :, e * 2 + fh,
                                        jh * 128 : (jh + 1) * 128,
                                    ],
                                    rhs=aggT[(e, fh)][:],
                                    start=(e == 0 and fh == 0),
                                    stop=(e == NE - 1 and fh == 1),
                                )
                        if has_bias:
                            nc.vector.scalar_tensor_tensor(
                                ph3[:], ph3[:], 1.0, hbt_sb[:, jh, :],
                                ALU.bypass, ALU.add,
                            )
                        nc.scalar.activation(h3q[:, jh, :], ph3[:], AF.Relu)

                if layer == 1:
                    # fc1 prefetch ring: lands during boundary-1 + layer 2
                    for blk in range(FC1_BUFS):
                        issue_fc1(blk)

            # ---- fc1: z_partial[1, 256], fp8 DoubleRow GEMV ----
            NPRE = FC1_BUFS + N_SCAV_A + N_SCAV_G
            pz = aggp.tile([1, H], f32, tag="agg", name="pz")
            for blk in range(FC1_NBLK):
                if blk < FC1_BUFS and NPRE + blk < FC1_NBLK:
                    issue_fc1(NPRE + blk)
                elif NPRE <= blk and blk + FC1_BUFS < FC1_NBLK:
                    issue_fc1(blk + FC1_BUFS)
                w = fc1_tiles[blk]
                for ch in range(NCH):
                    n = blk * NCH + ch
                    nc.tensor.matmul(
                        pz[:],
                        lhsT=h3q[:, :, n : n + 1],
                        rhs=w[:, ch],
                        start=(n == 0),
                        stop=(n == SHARD - 1),
                        perf_mode=DR,
                    )
            zsb = spool.tile([1, H], f32, tag="zsb")
            nc.vector.tensor_copy(zsb[:], pz[:])

            # AllGather per-core z partials, then sum over ranks on the PE
            # with a K=8 ones-matmul -- which also transposes z into the
            # [128, 1] column layout fc2 needs.
            agzin = dram.tile([1, H], f32, tag="agzin")
            nc.gpsimd.dma_start(agzin[:], zsb[:])
            agzout = dram.tile(
                [NCORES, 1, H], f32, tag="agzout", addr_space="Shared"
            )
            nc.gpsimd.collective_compute(
                "AllGather",
                ALU.bypass,
                replica_groups=rg,
                ins=[agzin.opt()],
                outs=[agzout.opt()],
            )
            zparts = spool.tile([NCORES, H], f32, tag="zparts")
            nc.sync.dma_start(zparts[:], agzout[:, 0, :])

            po = aggp.tile([C, 1], f32, tag="agg", name="po")
            for k in range(2):
                poz = aggp.tile([128, 1], f32, tag="agg", name=f"poz{k}")
                nc.tensor.matmul(
                    poz[:],
                    lhsT=zparts[:, k * 128 : (k + 1) * 128],
                    rhs=ones8[:],
                    start=True,
                    stop=True,
                )
                zr = spool.tile([128, 1], bf16, tag=f"zr{k}")
                nc.scalar.activation(
                    zr[:],
                    poz[:],
                    AF.Relu,
                    bias=fc1b_sb[:, k : k + 1],
                    scale=1.0 / (FC1_SCALE * H_SCALE),
                )
                nc.tensor.matmul(
                    po[:],
                    lhsT=fc2t_sb[:, k * C : (k + 1) * C],
                    rhs=zr[:],
                    start=(k == 0),
                    stop=(k == 1),
                )
            osb = spool.tile([C, 1], f32, tag="osb")
            nc.scalar.activation(osb[:], po[:], AF.Sigmoid, bias=fc2b_sb[:, 0:1])
            nc.gpsimd.dma_start(OUT[:], osb[:])

    nc.compile()
    _split_drain_waits(nc)
    return nc


def _prep_shared(feat, W0, b0, W1, b1, W2, b2, fc1_b, fc2_w, fc2_b):
    """Host layout prep for the tensors every core receives identically."""
    # g0[p, ch, i, f] = feat[ch*256 + i*128 + p, f] * H_SCALE
    g0 = np.ascontiguousarray(
        (feat * H_SCALE)
        .reshape(NCH, 2, 128, F_IN)
        .transpose(2, 0, 1, 3)
    ).astype(FP8)

    # wt[l, p, e*2+fh, j] = W_l[e][j, fh*128+p]
    wt = np.empty((3, 128, NE * 2, H), dtype=BF16)
    for li, W in enumerate((W0, W1, W2)):
        for e in range(NE):
            wte = np.ascontiguousarray(W[e].T).astype(BF16)  # [F, H]
            wt[li, :, e * 2 + 0, :] = wte[:128]
            wt[li, :, e * 2 + 1, :] = wte[128:]

    fc1b = np.ascontiguousarray(fc1_b.reshape(2, 128).T).astype(np.float32)
    fc2t = np.ascontiguousarray(
        fc2_w.T.reshape(2, 128, C).transpose(1, 0, 2).reshape(128, 2 * C)
    ).astype(BF16)
    fc2b = fc2_b.reshape(C, 1).astype(np.float32)
    return g0, wt, fc1b, fc2t, fc2b


def _prep_graph(edges):
    """Per-(etype, core) fp8 adjacency + drain corrections.

    A entry stored = cnt * q_d with q_d = fp8(A_SCALE/deg_d); corr so that
    q_d * corr_d == 1/deg_d. Chunk pairs packed in ORDER (consumption
    order: even chunks first, then odd).
    """
    atp = np.empty((NCORES, NE, NCHP, 128, 2, 2, SHARD), dtype=FP8)
    corr = np.empty((NCORES, 128, NE, SHARD), dtype=BF16)
    order = np.asarray(ORDER)
    for e in range(NE):
        src = np.asarray(edges[e, 0], dtype=np.int64)
        dst = np.asarray(edges[e, 1], dtype=np.int64)
        deg = np.bincount(dst, minlength=N_OBJ).astype(np.float64)
        q = (A_SCALE / np.maximum(deg, 1.0)).astype(FP8).astype(np.float32)
        cnt = (
            np.bincount(src * N_OBJ + dst, minlength=N_OBJ * N_OBJ)
            .reshape(N_OBJ, N_OBJ)
            .astype(np.float32)
        )
        a_store = (cnt * q[None, :]).astype(FP8)  # [src, dst]
        corr_e = np.where(
            deg > 0, 1.0 / (np.maximum(deg, 1.0) * q.astype(np.float64)), 0.0
        ).astype(np.float32)
        for c in range(NCORES):
            sl = a_store[:, c * SHARD : (c + 1) * SHARD]  # [4096, 512]
            # [src, d] -> [ch, i, p, d], reorder ch, -> [j, p, sub, i, d]
            atp[c, e] = (
                sl.reshape(NCH, 2, 128, SHARD)[order]
                .reshape(NCHP, 2, 2, 128, SHARD)
                .transpose(0, 3, 1, 2, 4)
            )
            corr[c, :, e, :] = corr_e[c * SHARD : (c + 1) * SHARD][None, :]
    return atp, corr


def _prep_bias(edges, b0, b1, b2):
    """hbias[l][dst, j] = sum_e ind_e[dst] * b_l[e][j] (H_SCALE domain)."""
    bs = np.stack([np.asarray(b0), np.asarray(b1), np.asarray(b2)])
    if not np.any(bs):
        return None
    hb = np.zeros((3, N_OBJ, H), dtype=np.float64)
    for e in range(NE):
        dst = np.asarray(edges[e, 1], dtype=np.int64)
        ind = (np.bincount(dst, minlength=N_OBJ) > 0).astype(np.float64)
        for li in range(3):
            hb[li] += ind[:, None] * bs[li, e][None, :]
    hb *= H_SCALE
    hbn = np.empty((NCORES, 128, 2, 4, H), dtype=BF16)
    hbt = np.empty((NCORES, 128, 2, SHARD), dtype=BF16)
    for c in range(NCORES):
        own = hb[:, c * SHARD : (c + 1) * SHARD, :]  # [3, 512, H]
        for li in range(2):
            hbn[c, :, li] = own[li].reshape(4, 128, H).transpose(1, 0, 2)
        hbt[c] = own[2].T.reshape(2, 128, SHARD).transpose(1, 0, 2)
    return hbn, hbt


def _prep_fc1(fc1_w):
    """Per-core column slice of fc1_w: [blk, p, ch, i, o] fp8, DoubleRow."""
    out = []
    ksl = SHARD * H  # 131072 flat positions per core
    for c in range(NCORES):
        sl = np.ascontiguousarray(fc1_w[:, c * ksl : (c + 1) * ksl].T)
        packed = np.ascontiguousarray(
            (sl * FC1_SCALE)
            .reshape(FC1_NBLK, NCH, 2, 128, H)
            .transpose(0, 3, 1, 2, 4)
        ).astype(FP8)
        out.append(packed)
    return out


def _make_in_maps(inputs):
    feat = np.asarray(inputs["feat"], dtype=np.float32)
    edges = np.asarray(inputs["edges"])
    g0, wt, fc1b, fc2t, fc2b = _prep_shared(
        feat,
        np.asarray(inputs["W0"]), np.asarray(inputs["b0"]),
        np.asarray(inputs["W1"]), np.asarray(inputs["b1"]),
        np.asarray(inputs["W2"]), np.asarray(inputs["b2"]),
        np.asarray(inputs["fc1_b"]), np.asarray(inputs["fc2_w"]),
        np.asarray(inputs["fc2_b"]),
    )
    atp, corr = _prep_graph(edges)
    fc1t = _prep_fc1(np.asarray(inputs["fc1_w"]))
    hbias = _prep_bias(edges, inputs["b0"], inputs["b1"], inputs["b2"])
    maps = []
    for c in range(NCORES):
        m = {
            "g0": g0, "atp": atp[c], "corr": corr[c], "wt": wt,
            "fc1t": fc1t[c], "fc1b": fc1b, "fc2t": fc2t, "fc2b": fc2b,
        }
        if hbias is not None:
            m["hbn"] = hbias[0][c]
            m["hbt"] = hbias[1][c]
        maps.append(m)
    return maps, hbias is not None


def kernel(feat, edges, W0, b0, W1, b1, W2, b2, fc1_w, fc1_b, fc2_w, fc2_b):
    from concourse.bass_utils import run_bass_kernel_spmd

    in_maps, has_bias = _make_in_maps(
        dict(
            feat=feat, edges=edges, W0=W0, b0=b0, W1=W1, b1=b1, W2=W2, b2=b2,
            fc1_w=fc1_w, fc1_b=fc1_b, fc2_w=fc2_w, fc2_b=fc2_b,
        )
    )
    key = ("nc", has_bias)
    if key not in _BASS_CACHE:
        _BASS_CACHE[key] = _build_bass(has_bias=has_bias)
    nc = _BASS_CACHE[key]

    res = run_bass_kernel_spmd(nc, in_maps, core_ids=list(range(NCORES)))
    out = np.asarray(res.results[0]["out"]).reshape(C)
    return out.astype(np.float32)


def run_profiled(inputs, trace_cores=None):
    """Test-only: run with NTFF tracing; returns BassKernelResults."""
    from concourse import bass_utils
    from concourse.bass_utils import run_bass_kernel_spmd

    bass_utils.upload_artifacts = lambda tmpdir: f"local://{tmpdir}"
    in_maps, has_bias = _make_in_maps(inputs)
    key = ("nc", has_bias)
    if key not in _BASS_CACHE:
        _BASS_CACHE[key] = _build_bass(has_bias=has_bias)
    nc = _BASS_CACHE[key]
    tmpdir = "/tmp/gcn_profile"
    import shutil, os
    shutil.rmtree(tmpdir, ignore_errors=True)
    os.makedirs(tmpdir, exist_ok=True)
    return run_bass_kernel_spmd(
        nc,
        in_maps,
        core_ids=list(range(NCORES)),
        trace=True,
        tmpdir=tmpdir,
        trace_cores=trace_cores,
    )
